# revision 17
# baseline (speedup 1.0000x reference)
"""Causal single-head attention on 8 Trainium2 NeuronCores.

Reference computation (per batch b of 16):
    q = x @ Wq; k = x @ Wk; v = x @ Wv        # x [2048, 512], W* [512, 64]
    out = softmax_causal(q @ k.T / 8) @ v     # out [2048, 64]

Sharding: data-parallel over batch, 2 batches per core, weights replicated.

Per-core kernel (batch-local b in {0,1}), all matmul operands bf16
(psum accumulation fp32; rel-err budget 2e-2 leaves ample margin):
  - host supplies xT = x[b].T in bf16 so the D-contraction sits on
    partitions; xt DMAs issue from the gpsimd queue (parallel to the
    const DMAs on sync) with the Q0 slices first, so the first
    projection starts ~15us earlier than a single-queue issue.
  - qT/kT: psum[0:64]=qT, psum[64:128]=kT via packed lhsT [Wq|Wk]
  - qklo tile = swapped halves of qk (kT at 0:64, qT at 64:128) via
    two SBUF->SBUF DMAs; gives both partition homes needed below.
  - scores TRANSPOSED ST[k, q] so softmax's denominator is a
    partition-dim sum the PV matmul computes via a ones column.
  - ST contraction is only K=64, so even/odd k-blocks run CONCURRENTLY
    in the PE array via 2-way row tiling (tile_position (0,0)/(64,0)):
      even j: lhsT=qklo[0:64](kT),  moving qk[0:64](qT)
      odd  j: lhsT=qk[64:128](kT),  moving qklo[64:128](qT)
    ~2x on the score matmuls.
  - vT via Wv-stationary matmuls; v natural layout produced by the
    DMA-transpose XBAR (16x128 bf16 tiles) straight into v1[:, j, 0:64]
    (no PE transposes); v1[:, :, 64] = 1 makes PV emit the denominator.
  - exp on ACT out of psum in [128, 1024] chunks (pair of k-blocks),
    junk prefix before the first computed column sliced off; output
    bf16 feeds PV directly.
  - causal: k-blocks above the diagonal skipped; diagonal blocks get a
    triangular mask multiply (on the otherwise-idle gpsimd engine) and
    suffix-sliced matmuls.
  - oT[65, 512] accumulates [v|1].T @ p~ over k-blocks in psum; row 64
    is the denominator l; out = oT[0:64] * bcast(1/l) via DVE
    reciprocal_approx_fast (reading psum directly) + gpsimd
    partition_broadcast.
  - projection matmuls of chunk Q+1 are interleaved between attention
    pairs of chunk Q in emission order, so the in-order PE stream fills
    the ACT-gated stalls of the attention inner loop.
  - output written transposed [2, 64, 2048] fp32; host transposes back.
"""

import sys

sys.path.insert(0, "/opt/trn_rl_repo")

import numpy as np

B, T, D, HD = 16, 2048, 512, 64
NCORES = 8
BPC = B // NCORES          # batches per core
NQ = T // 512              # 512-wide q chunks per batch
NJ = T // 128              # 128-wide k blocks per batch
ND = D // 128              # 128-deep contraction tiles

_cache = {}


def _build_nc():
    import concourse.bacc as bacc
    import concourse.mybir as mybir
    import concourse.tile as tile

    F32 = mybir.dt.float32
    BF16 = mybir.dt.bfloat16
    AF = mybir.ActivationFunctionType

    nc = bacc.Bacc("TRN2", target_bir_lowering=False, debug=False)

    xt_d = nc.dram_tensor("xt", [BPC, D, T], BF16, kind="ExternalInput")
    wqk_d = nc.dram_tensor("wqk", [ND, 128, 128], BF16, kind="ExternalInput")
    wv_d = nc.dram_tensor("wv", [ND, 128, HD], BF16, kind="ExternalInput")
    mneg_d = nc.dram_tensor("mneg", [128, 128], BF16, kind="ExternalInput")
    identb_d = nc.dram_tensor("identb", [128, 128], BF16, kind="ExternalInput")
    ident_d = nc.dram_tensor("ident", [64, 64], F32, kind="ExternalInput")
    ot_d = nc.dram_tensor("ot", [BPC, HD, T], F32, kind="ExternalOutput")

    with tile.TileContext(nc) as tc:
        with (
            tc.tile_pool(name="const", bufs=1) as cpool,
            tc.tile_pool(name="xt", bufs=1) as xtpool,
            tc.tile_pool(name="qk", bufs=2) as qkpool,
            tc.tile_pool(name="qklo", bufs=2) as qklopool,
            tc.tile_pool(name="vt", bufs=2) as vtpool,
            tc.tile_pool(name="v1", bufs=2) as v1pool,
            tc.tile_pool(name="pt", bufs=4) as ptpool,
            tc.tile_pool(name="rec", bufs=2) as recpool,
            tc.tile_pool(name="outp", bufs=2) as outpool,
            tc.tile_pool(name="st", bufs=2, space="PSUM") as stpool,
            tc.tile_pool(name="otp", bufs=2, space="PSUM") as otpool,
            tc.tile_pool(name="aux", bufs=2, space="PSUM") as auxpool,
        ):
            # ---- constants / weights: issued on the scalar (ACT) HWDGE
            # queue so the sync queue's first entries are the qklo shifts
            # the first attention chunk waits on; wqk/wv land as single
            # combined DMAs (5 configs instead of 11 at ~610ns each) ----
            wqk_all = cpool.tile([128, ND, 128], BF16, tag="wqk")
            nc.scalar.dma_start(wqk_all[:], wqk_d[:].rearrange("d p c -> p d c"))
            wqk = [wqk_all[:, d, :] for d in range(ND)]
            wv_all = cpool.tile([128, ND, HD], BF16, tag="wv")
            nc.scalar.dma_start(wv_all[:], wv_d[:].rearrange("d p c -> p d c"))
            wv = [wv_all[:, d, :] for d in range(ND)]
            ident = cpool.tile([64, 64], F32, tag="ident")
            nc.scalar.dma_start(ident[:], ident_d[:])
            mneg = cpool.tile([128, 128], BF16, tag="mneg")
            nc.scalar.dma_start(mneg[:], mneg_d[:])
            identb = cpool.tile([128, 128], BF16, tag="identb")
            nc.scalar.dma_start(identb[:], identb_d[:])

            # warm the exp table set on ACT while the first loads run
            scratch = cpool.tile([1, 1], F32, tag="scratch")
            nc.scalar.activation(scratch[:], scratch[:], AF.Exp)

            # PE clock warm-up: dependency-free junk matmuls ramp the
            # tensor engine out of its low p-state (0.65 -> 2.4 GHz needs
            # ~3us of continuous execution) while the x DMAs are in flight,
            # so proj(0,0) runs at full clock instead of 3.7x slower.
            wscr = cpool.tile([128, 512], BF16, tag="wscr")
            nc.gpsimd.memset(wscr[:], 0.5)
            for i in range(5):
                pw = auxpool.tile([128, 512], F32, tag="aux", name="pwarm")
                nc.tensor.matmul(
                    pw[:], wscr[:, 0:128], wscr[:], start=True, stop=True
                )

            # ---- x loads (gpsimd queue, Q0 slices first) ----
            xts = {}
            for b in range(BPC):
                for d in range(ND):
                    xts[(b, d)] = xtpool.tile(
                        [128, T], BF16, tag=f"xt{b}{d}", name=f"xt{b}{d}"
                    )
            for d in range(ND):
                nc.gpsimd.dma_start(
                    xts[(0, d)][:, 0:512], xt_d[0, 128 * d:128 * (d + 1), 0:512]
                )
            for d in range(ND):
                nc.gpsimd.dma_start(
                    xts[(0, d)][:, 512:T], xt_d[0, 128 * d:128 * (d + 1), 512:T]
                )
            def emit_b1_xt():
                for d in range(ND):
                    nc.gpsimd.dma_start(
                        xts[(1, d)][:, :], xt_d[1, 128 * d:128 * (d + 1), :]
                    )

            qks, qklos, v1s, vts = {}, {}, {}, {}
            for b in range(BPC):
                qks[b] = qkpool.tile([128, T], BF16, tag="qk", name=f"qk{b}")
                qklos[b] = qklopool.tile([128, T], BF16, tag="qklo", name=f"qklo{b}")
                v1s[b] = v1pool.tile([128, NJ, HD + 1], BF16, tag="v1", name=f"v1{b}")
                vts[b] = vtpool.tile([64, T], F32, tag="vt", name=f"vt{b}")
            # ones column for the PV denominator row, after the critical
            # xt configs in the gpsimd stream
            for b in range(BPC):
                nc.gpsimd.memset(v1s[b][:, :, HD:HD + 1], 1.0)

            def proj_steps(b, Q):
                """Emission steps for everything attention chunk (b, Q) needs
                from tokens [512Q, 512Q+512): returns a list of callables so
                the caller can interleave them between attention pairs."""
                s = slice(512 * Q, 512 * (Q + 1))
                qk, qklo, v1, vt = qks[b], qklos[b], v1s[b], vts[b]
                state = {}

                def mk_qk(d):
                    def f():
                        if d == 0:
                            state["pqk"] = auxpool.tile(
                                [128, 512], F32, tag="aux", name="pqk"
                            )
                        nc.tensor.matmul(
                            state["pqk"][:], wqk[d], xts[(b, d)][:, s],
                            start=(d == 0), stop=(d == ND - 1),
                        )
                        if d == ND - 1:
                            nc.vector.tensor_copy(qk[:, s], state["pqk"][:])
                            # swapped halves: kT to 0:64, qT to 64:128
                            nc.sync.dma_start(qklo[0:64, s], qk[64:128, s])
                            nc.scalar.dma_start(qklo[64:128, s], qk[0:64, s])
                    return f

                def mk_v(d):
                    def f():
                        if d == 0:
                            state["pv"] = auxpool.tile(
                                [64, 512], F32, tag="aux", name="pv"
                            )
                        nc.tensor.matmul(
                            state["pv"][:], wv[d], xts[(b, d)][:, s],
                            start=(d == 0), stop=(d == ND - 1),
                        )
                        if d == ND - 1:
                            nc.vector.tensor_copy(vt[:, s], state["pv"][:])
                            for t2 in range(2 * Q, 2 * Q + 2):
                                p2 = auxpool.tile(
                                    [128, 128], F32, tag="aux", name="ptr"
                                )
                                for tt in range(2):
                                    nc.tensor.transpose(
                                        p2[:, 64 * tt:64 * (tt + 1)],
                                        vt[:, 128 * (2 * t2 + tt):
                                           128 * (2 * t2 + tt + 1)],
                                        ident[:],
                                    )
                                nc.vector.tensor_copy(
                                    v1[:, 2 * t2:2 * t2 + 2, 0:HD],
                                    p2[:].rearrange("p (a c) -> p a c", a=2),
                                )
                    return f

                return [mk_qk(d) for d in range(ND)] + [mk_v(d) for d in range(ND)]

            def emit_attn_q(b, Q, fillers):
                """One query chunk: all causal k-blocks in even/odd pairs
                run concurrently via 2-way PE row tiling; PV skewed one
                pair behind ST; proj steps for the next chunk interleaved."""
                qk, qklo, v1 = qks[b], qklos[b], v1s[b]
                pot = otpool.tile([HD + 1, 512], F32, tag="ot", name="pot")
                njb = 4 * (Q + 1)
                jlast = njb - 1

                def emit_pv(p_tile, g):
                    je, jo = 2 * g, 2 * g + 1
                    w0e = 128 * (je - 4 * Q) if je >= 4 * Q else 0
                    w0o = 128 * (jo - 4 * Q) if jo >= 4 * Q else 0
                    nc.tensor.matmul(
                        pot[:, w0e:512], v1[:, je, :], p_tile[:, w0e:512],
                        start=(je == 0), stop=False,
                    )
                    nc.tensor.matmul(
                        pot[:, w0o:512], v1[:, jo, :],
                        p_tile[:, 512 + w0o:1024],
                        start=False, stop=(jo == jlast),
                    )

                pending = []
                for g in range(njb // 2):
                    je, jo = 2 * g, 2 * g + 1
                    w0e = 128 * (je - 4 * Q) if je >= 4 * Q else 0
                    w0o = 128 * (jo - 4 * Q) if jo >= 4 * Q else 0
                    diag_e, diag_o = je >= 4 * Q, jo >= 4 * Q
                    pst = stpool.tile([128, 1024], F32, tag="st", name="pst")
                    # even block: kT stationary on partitions 0:64 (rg 0)
                    nc.tensor.matmul(
                        pst[:, w0e:512],
                        qklo[0:64, 128 * je:128 * (je + 1)],
                        qk[0:64, 512 * Q + w0e:512 * (Q + 1)],
                        start=True, stop=not diag_e,
                    )
                    # odd block: kT stationary on partitions 64:128 (rg 64)
                    nc.tensor.matmul(
                        pst[:, 512 + w0o:1024],
                        qk[64:128, 128 * jo:128 * (jo + 1)],
                        qklo[64:128, 512 * Q + w0o:512 * (Q + 1)],
                        start=True, stop=not diag_o,
                    )
                    # diagonal blocks: add -1e5 above the diagonal ON the PE
                    # (I.T @ mneg accumulated into the 128-wide diag window),
                    # so exp underflows to exactly 0 and no cross-engine mask
                    # op sits between exp and PV
                    if diag_e:
                        nc.tensor.matmul(
                            pst[:, w0e:w0e + 128], identb[:], mneg[:],
                            start=False, stop=True,
                        )
                    if diag_o:
                        nc.tensor.matmul(
                            pst[:, 512 + w0o:512 + w0o + 128], identb[:],
                            mneg[:], start=False, stop=True,
                        )
                    if fillers:
                        fillers.pop(0)()
                    ptil = ptpool.tile([128, 1024], BF16, tag="pt", name="ptil")
                    nc.scalar.activation(
                        ptil[:, w0e:1024], pst[:, w0e:1024], AF.Exp,
                        scale=1.0 / np.sqrt(HD),
                    )
                    # PV runs two pairs behind ST so the in-order PE stream
                    # never waits on the exp of the pair it just issued
                    pending.append((ptil, g))
                    if len(pending) > 2:
                        emit_pv(*pending.pop(0))
                for p in pending:
                    if fillers:
                        fillers.pop(0)()
                    emit_pv(*p)
                while fillers:
                    fillers.pop(0)()

                # normalize: out = oT[0:64] * broadcast(1 / l), PE-free
                # (reciprocal_approx_fast is a custom-DVE op that mishandles
                #  psum APs at a nonzero base partition — stage l via SBUF)
                lsb = recpool.tile([1, 512], F32, tag="lsb", name="lsb")
                nc.vector.tensor_copy(lsb[:], pot[HD:HD + 1, :])
                rsb = recpool.tile([1, 512], F32, tag="rsb", name="rsb")
                nc.vector.reciprocal_approx_fast(rsb[:], lsb[:])
                rcb = outpool.tile([HD, 512], F32, tag="rcb", name="rcb")
                nc.gpsimd.partition_broadcast(rcb[:], rsb[:])
                osb = outpool.tile([HD, 512], F32, tag="out", name="osb")
                nc.vector.tensor_mul(osb[:], pot[0:HD, :], rcb[:])
                nc.sync.dma_start(
                    ot_d[b, :, 512 * Q:512 * (Q + 1)], osb[:]
                )

            # ---- emission schedule (no fillers bisect) ----
            import os
            if os.environ.get("KNOFILL"):
                for b in range(BPC):
                    if b == 1:
                        emit_b1_xt()
                    for Q in range(NQ):
                        for f in proj_steps(b, Q):
                            f()
                        emit_attn_q(b, Q, [])
            else:
                for f in proj_steps(0, 0):
                    f()
                emit_attn_q(0, 0, proj_steps(0, 1))
                emit_attn_q(0, 1, proj_steps(0, 2))
                emit_b1_xt()
                emit_attn_q(0, 2, proj_steps(0, 3))
                emit_attn_q(0, 3, proj_steps(1, 0))
                emit_attn_q(1, 0, proj_steps(1, 1))
                emit_attn_q(1, 1, proj_steps(1, 2))
                emit_attn_q(1, 2, proj_steps(1, 3))
                emit_attn_q(1, 3, [])

    nc.compile()
    return nc


def _get_nc():
    if "nc" not in _cache:
        _cache["nc"] = _build_nc()
    return _cache["nc"]


def kernel(x, Wq, Wk, Wv, _trace=False, _trace_kwargs=None):
    import ml_dtypes
    from concourse.bass_utils import run_bass_kernel_spmd

    bf16 = ml_dtypes.bfloat16
    x = np.asarray(x, dtype=np.float32)
    Wq = np.asarray(Wq, dtype=np.float32)
    Wk = np.asarray(Wk, dtype=np.float32)
    Wv = np.asarray(Wv, dtype=np.float32)

    nc = _get_nc()

    wqk = np.ascontiguousarray(
        np.concatenate([Wq, Wk], axis=1).reshape(ND, 128, 128)
    ).astype(bf16)
    wv = np.ascontiguousarray(Wv.reshape(ND, 128, HD)).astype(bf16)
    mneg = np.where(
        np.arange(128)[None, :] < np.arange(128)[:, None], -1e5, 0.0
    ).astype(np.float32).astype(bf16)
    identb = np.eye(128, dtype=np.float32).astype(bf16)
    ident = np.eye(64, dtype=np.float32)

    in_maps = []
    for c in range(NCORES):
        xt = np.ascontiguousarray(
            x[BPC * c:BPC * (c + 1)].transpose(0, 2, 1)
        ).astype(bf16)
        in_maps.append(
            {
                "xt": xt,
                "wqk": wqk,
                "wv": wv,
                "mneg": mneg,
                "identb": identb,
                "ident": ident,
            }
        )

    kwargs = dict(_trace_kwargs or {})
    res = run_bass_kernel_spmd(
        nc, in_maps, list(range(NCORES)), trace=_trace, **kwargs
    )

    out = np.empty((B, T, HD), dtype=np.float32)
    for c in range(NCORES):
        ot = res.results[c]["ot"]  # [BPC, HD, T]
        out[BPC * c:BPC * (c + 1)] = ot.transpose(0, 2, 1)
    if _trace:
        _cache["last_results"] = res
    return out


# revision 18
# speedup vs baseline: 1.0020x; 1.0020x over previous
"""Causal single-head attention on 8 Trainium2 NeuronCores.

Reference computation (per batch b of 16):
    q = x @ Wq; k = x @ Wk; v = x @ Wv        # x [2048, 512], W* [512, 64]
    out = softmax_causal(q @ k.T / 8) @ v     # out [2048, 64]

Sharding: data-parallel over batch, 2 batches per core, weights replicated.

Per-core kernel (batch-local b in {0,1}), all matmul operands bf16
(psum accumulation fp32; rel-err budget 2e-2 leaves ample margin):
  - host supplies xT = x[b].T in bf16 so the D-contraction sits on
    partitions; xt DMAs issue from the gpsimd queue (parallel to the
    const DMAs on sync) with the Q0 slices first, so the first
    projection starts ~15us earlier than a single-queue issue.
  - qT/kT: psum[0:64]=qT, psum[64:128]=kT via packed lhsT [Wq|Wk]
  - qklo tile = swapped halves of qk (kT at 0:64, qT at 64:128) via
    two SBUF->SBUF DMAs; gives both partition homes needed below.
  - scores TRANSPOSED ST[k, q] so softmax's denominator is a
    partition-dim sum the PV matmul computes via a ones column.
  - ST contraction is only K=64, so even/odd k-blocks run CONCURRENTLY
    in the PE array via 2-way row tiling (tile_position (0,0)/(64,0)):
      even j: lhsT=qklo[0:64](kT),  moving qk[0:64](qT)
      odd  j: lhsT=qk[64:128](kT),  moving qklo[64:128](qT)
    ~2x on the score matmuls.
  - vT via Wv-stationary matmuls; v natural layout produced by the
    DMA-transpose XBAR (16x128 bf16 tiles) straight into v1[:, j, 0:64]
    (no PE transposes); v1[:, :, 64] = 1 makes PV emit the denominator.
  - exp on ACT out of psum in [128, 1024] chunks (pair of k-blocks),
    junk prefix before the first computed column sliced off; output
    bf16 feeds PV directly.
  - causal: k-blocks above the diagonal skipped; diagonal blocks get a
    triangular mask multiply (on the otherwise-idle gpsimd engine) and
    suffix-sliced matmuls.
  - oT[65, 512] accumulates [v|1].T @ p~ over k-blocks in psum; row 64
    is the denominator l; out = oT[0:64] * bcast(1/l) via DVE
    reciprocal_approx_fast (reading psum directly) + gpsimd
    partition_broadcast.
  - projection matmuls of chunk Q+1 are interleaved between attention
    pairs of chunk Q in emission order, so the in-order PE stream fills
    the ACT-gated stalls of the attention inner loop.
  - output written transposed [2, 64, 2048] fp32; host transposes back.
"""

import sys

sys.path.insert(0, "/opt/trn_rl_repo")

import numpy as np

B, T, D, HD = 16, 2048, 512, 64
NCORES = 8
BPC = B // NCORES          # batches per core
NQ = T // 512              # 512-wide q chunks per batch
NJ = T // 128              # 128-wide k blocks per batch
ND = D // 128              # 128-deep contraction tiles

_cache = {}


def _build_nc():
    import concourse.bacc as bacc
    import concourse.mybir as mybir
    import concourse.tile as tile

    F32 = mybir.dt.float32
    BF16 = mybir.dt.bfloat16
    AF = mybir.ActivationFunctionType

    nc = bacc.Bacc("TRN2", target_bir_lowering=False, debug=False)

    xt_d = nc.dram_tensor("xt", [BPC, D, T], BF16, kind="ExternalInput")
    wqk_d = nc.dram_tensor("wqk", [ND, 128, 128], BF16, kind="ExternalInput")
    wv_d = nc.dram_tensor("wv", [ND, 128, HD], BF16, kind="ExternalInput")
    mneg_d = nc.dram_tensor("mneg", [128, 128], BF16, kind="ExternalInput")
    identb_d = nc.dram_tensor("identb", [128, 128], BF16, kind="ExternalInput")
    ident_d = nc.dram_tensor("ident", [64, 64], F32, kind="ExternalInput")
    onescol_d = nc.dram_tensor("onescol", [128, NJ], BF16, kind="ExternalInput")
    ot_d = nc.dram_tensor("ot", [BPC, HD, T], F32, kind="ExternalOutput")

    with tile.TileContext(nc) as tc:
        with (
            tc.tile_pool(name="const", bufs=1) as cpool,
            tc.tile_pool(name="xt", bufs=1) as xtpool,
            tc.tile_pool(name="qk", bufs=2) as qkpool,
            tc.tile_pool(name="qklo", bufs=2) as qklopool,
            tc.tile_pool(name="vt", bufs=2) as vtpool,
            tc.tile_pool(name="v1", bufs=2) as v1pool,
            tc.tile_pool(name="pt", bufs=4) as ptpool,
            tc.tile_pool(name="rec", bufs=2) as recpool,
            tc.tile_pool(name="outp", bufs=2) as outpool,
            tc.tile_pool(name="st", bufs=2, space="PSUM") as stpool,
            tc.tile_pool(name="otp", bufs=2, space="PSUM") as otpool,
            tc.tile_pool(name="aux", bufs=2, space="PSUM") as auxpool,
        ):
            # ---- constants / weights: issued on the scalar (ACT) HWDGE
            # queue so the sync queue's first entries are the qklo shifts
            # the first attention chunk waits on; wqk/wv land as single
            # combined DMAs (5 configs instead of 11 at ~610ns each) ----
            wqk_all = cpool.tile([128, ND, 128], BF16, tag="wqk")
            nc.scalar.dma_start(wqk_all[:], wqk_d[:].rearrange("d p c -> p d c"))
            wqk = [wqk_all[:, d, :] for d in range(ND)]
            wv_all = cpool.tile([128, ND, HD], BF16, tag="wv")
            nc.scalar.dma_start(wv_all[:], wv_d[:].rearrange("d p c -> p d c"))
            wv = [wv_all[:, d, :] for d in range(ND)]
            ident = cpool.tile([64, 64], F32, tag="ident")
            nc.scalar.dma_start(ident[:], ident_d[:])
            mneg = cpool.tile([128, 128], BF16, tag="mneg")
            nc.scalar.dma_start(mneg[:], mneg_d[:])
            identb = cpool.tile([128, 128], BF16, tag="identb")
            nc.scalar.dma_start(identb[:], identb_d[:])
            onescol = cpool.tile([128, NJ], BF16, tag="onescol")
            nc.scalar.dma_start(onescol[:], onescol_d[:])

            # warm the exp table set on ACT while the first loads run
            scratch = cpool.tile([1, 1], F32, tag="scratch")
            nc.scalar.activation(scratch[:], scratch[:], AF.Exp)

            # PE clock warm-up: dependency-free junk matmuls ramp the
            # tensor engine out of its low p-state (0.65 -> 2.4 GHz needs
            # ~3us of continuous execution) while the x DMAs are in flight,
            # so proj(0,0) runs at full clock instead of 3.7x slower.
            wscr = cpool.tile([128, 512], BF16, tag="wscr")
            nc.gpsimd.memset(wscr[:], 0.5)
            for i in range(5):
                pw = auxpool.tile([128, 512], F32, tag="aux", name="pwarm")
                nc.tensor.matmul(
                    pw[:], wscr[:, 0:128], wscr[:], start=True, stop=True
                )

            # ---- x loads (gpsimd queue, Q0 slices first) ----
            xts = {}
            for b in range(BPC):
                for d in range(ND):
                    xts[(b, d)] = xtpool.tile(
                        [128, T], BF16, tag=f"xt{b}{d}", name=f"xt{b}{d}"
                    )
            for d in range(ND):
                nc.gpsimd.dma_start(
                    xts[(0, d)][:, 0:512], xt_d[0, 128 * d:128 * (d + 1), 0:512]
                )
            for d in range(ND):
                nc.gpsimd.dma_start(
                    xts[(0, d)][:, 512:T], xt_d[0, 128 * d:128 * (d + 1), 512:T]
                )
            def emit_b1_xt():
                for d in range(ND):
                    nc.gpsimd.dma_start(
                        xts[(1, d)][:, :], xt_d[1, 128 * d:128 * (d + 1), :]
                    )

            qks, qklos, v1s, vts = {}, {}, {}, {}
            for b in range(BPC):
                qks[b] = qkpool.tile([128, T], BF16, tag="qk", name=f"qk{b}")
                qklos[b] = qklopool.tile([128, T], BF16, tag="qklo", name=f"qklo{b}")
                v1s[b] = v1pool.tile([128, NJ, HD + 1], BF16, tag="v1", name=f"v1{b}")
                vts[b] = vtpool.tile([64, T], F32, tag="vt", name=f"vt{b}")
            for b in range(BPC):
                nc.vector.tensor_copy(
                    v1s[b][:, :, HD:HD + 1],
                    onescol[:].rearrange("p (a c) -> p a c", c=1),
                )

            def proj_steps(b, Q):
                """Emission steps for everything attention chunk (b, Q) needs
                from tokens [512Q, 512Q+512): returns a list of callables so
                the caller can interleave them between attention pairs."""
                s = slice(512 * Q, 512 * (Q + 1))
                qk, qklo, v1, vt = qks[b], qklos[b], v1s[b], vts[b]
                state = {}

                def mk_qk(d):
                    def f():
                        if d == 0:
                            state["pqk"] = auxpool.tile(
                                [128, 512], F32, tag="aux", name="pqk"
                            )
                        nc.tensor.matmul(
                            state["pqk"][:], wqk[d], xts[(b, d)][:, s],
                            start=(d == 0), stop=(d == ND - 1),
                        )
                        if d == ND - 1:
                            nc.vector.tensor_copy(qk[:, s], state["pqk"][:])
                            # swapped halves: kT to 0:64, qT to 64:128
                            nc.sync.dma_start(qklo[0:64, s], qk[64:128, s])
                            nc.scalar.dma_start(qklo[64:128, s], qk[0:64, s])
                    return f

                def mk_v(d):
                    def f():
                        if d == 0:
                            state["pv"] = auxpool.tile(
                                [64, 512], F32, tag="aux", name="pv"
                            )
                        nc.tensor.matmul(
                            state["pv"][:], wv[d], xts[(b, d)][:, s],
                            start=(d == 0), stop=(d == ND - 1),
                        )
                        if d == ND - 1:
                            nc.vector.tensor_copy(vt[:, s], state["pv"][:])
                            for t2 in range(2 * Q, 2 * Q + 2):
                                p2 = auxpool.tile(
                                    [128, 128], F32, tag="aux", name="ptr"
                                )
                                for tt in range(2):
                                    nc.tensor.transpose(
                                        p2[:, 64 * tt:64 * (tt + 1)],
                                        vt[:, 128 * (2 * t2 + tt):
                                           128 * (2 * t2 + tt + 1)],
                                        ident[:],
                                    )
                                nc.vector.tensor_copy(
                                    v1[:, 2 * t2:2 * t2 + 2, 0:HD],
                                    p2[:].rearrange("p (a c) -> p a c", a=2),
                                )
                    return f

                return [mk_qk(d) for d in range(ND)] + [mk_v(d) for d in range(ND)]

            def emit_attn_q(b, Q, fillers):
                """One query chunk: all causal k-blocks in even/odd pairs
                run concurrently via 2-way PE row tiling; PV skewed one
                pair behind ST; proj steps for the next chunk interleaved."""
                qk, qklo, v1 = qks[b], qklos[b], v1s[b]
                pot = otpool.tile([HD + 1, 512], F32, tag="ot", name="pot")
                njb = 4 * (Q + 1)
                jlast = njb - 1

                def emit_pv(p_tile, g):
                    je, jo = 2 * g, 2 * g + 1
                    w0e = 128 * (je - 4 * Q) if je >= 4 * Q else 0
                    w0o = 128 * (jo - 4 * Q) if jo >= 4 * Q else 0
                    nc.tensor.matmul(
                        pot[:, w0e:512], v1[:, je, :], p_tile[:, w0e:512],
                        start=(je == 0), stop=False,
                    )
                    nc.tensor.matmul(
                        pot[:, w0o:512], v1[:, jo, :],
                        p_tile[:, 512 + w0o:1024],
                        start=False, stop=(jo == jlast),
                    )

                pending = []
                for g in range(njb // 2):
                    je, jo = 2 * g, 2 * g + 1
                    w0e = 128 * (je - 4 * Q) if je >= 4 * Q else 0
                    w0o = 128 * (jo - 4 * Q) if jo >= 4 * Q else 0
                    diag_e, diag_o = je >= 4 * Q, jo >= 4 * Q
                    pst = stpool.tile([128, 1024], F32, tag="st", name="pst")
                    # even block: kT stationary on partitions 0:64 (rg 0)
                    nc.tensor.matmul(
                        pst[:, w0e:512],
                        qklo[0:64, 128 * je:128 * (je + 1)],
                        qk[0:64, 512 * Q + w0e:512 * (Q + 1)],
                        start=True, stop=not diag_e,
                    )
                    # odd block: kT stationary on partitions 64:128 (rg 64)
                    nc.tensor.matmul(
                        pst[:, 512 + w0o:1024],
                        qk[64:128, 128 * jo:128 * (jo + 1)],
                        qklo[64:128, 512 * Q + w0o:512 * (Q + 1)],
                        start=True, stop=not diag_o,
                    )
                    # diagonal blocks: add -1e5 above the diagonal ON the PE
                    # (I.T @ mneg accumulated into the 128-wide diag window),
                    # so exp underflows to exactly 0 and no cross-engine mask
                    # op sits between exp and PV
                    if diag_e:
                        nc.tensor.matmul(
                            pst[:, w0e:w0e + 128], identb[:], mneg[:],
                            start=False, stop=True,
                        )
                    if diag_o:
                        nc.tensor.matmul(
                            pst[:, 512 + w0o:512 + w0o + 128], identb[:],
                            mneg[:], start=False, stop=True,
                        )
                    if fillers:
                        fillers.pop(0)()
                    ptil = ptpool.tile([128, 1024], BF16, tag="pt", name="ptil")
                    nc.scalar.activation(
                        ptil[:, w0e:1024], pst[:, w0e:1024], AF.Exp,
                        scale=1.0 / np.sqrt(HD),
                    )
                    # PV runs two pairs behind ST so the in-order PE stream
                    # never waits on the exp of the pair it just issued
                    pending.append((ptil, g))
                    if len(pending) > 2:
                        emit_pv(*pending.pop(0))
                for p in pending:
                    if fillers:
                        fillers.pop(0)()
                    emit_pv(*p)
                while fillers:
                    fillers.pop(0)()

                # normalize: out = oT[0:64] * broadcast(1 / l), PE-free
                # (reciprocal_approx_fast is a custom-DVE op that mishandles
                #  psum APs at a nonzero base partition — stage l via SBUF)
                lsb = recpool.tile([1, 512], F32, tag="lsb", name="lsb")
                nc.vector.tensor_copy(lsb[:], pot[HD:HD + 1, :])
                rsb = recpool.tile([1, 512], F32, tag="rsb", name="rsb")
                nc.vector.reciprocal_approx_fast(rsb[:], lsb[:])
                rcb = outpool.tile([HD, 512], F32, tag="rcb", name="rcb")
                nc.gpsimd.partition_broadcast(rcb[:], rsb[:])
                osb = outpool.tile([HD, 512], F32, tag="out", name="osb")
                nc.vector.tensor_mul(osb[:], pot[0:HD, :], rcb[:])
                nc.sync.dma_start(
                    ot_d[b, :, 512 * Q:512 * (Q + 1)], osb[:]
                )

            # ---- emission schedule (no fillers bisect) ----
            import os
            if os.environ.get("KNOFILL"):
                for b in range(BPC):
                    if b == 1:
                        emit_b1_xt()
                    for Q in range(NQ):
                        for f in proj_steps(b, Q):
                            f()
                        emit_attn_q(b, Q, [])
            else:
                for f in proj_steps(0, 0):
                    f()
                emit_attn_q(0, 0, proj_steps(0, 1))
                emit_attn_q(0, 1, proj_steps(0, 2))
                emit_b1_xt()
                emit_attn_q(0, 2, proj_steps(0, 3))
                emit_attn_q(0, 3, proj_steps(1, 0))
                emit_attn_q(1, 0, proj_steps(1, 1))
                emit_attn_q(1, 1, proj_steps(1, 2))
                emit_attn_q(1, 2, proj_steps(1, 3))
                emit_attn_q(1, 3, [])

    nc.compile()
    return nc


def _get_nc():
    if "nc" not in _cache:
        _cache["nc"] = _build_nc()
    return _cache["nc"]


def kernel(x, Wq, Wk, Wv, _trace=False, _trace_kwargs=None):
    import ml_dtypes
    from concourse.bass_utils import run_bass_kernel_spmd

    bf16 = ml_dtypes.bfloat16
    x = np.asarray(x, dtype=np.float32)
    Wq = np.asarray(Wq, dtype=np.float32)
    Wk = np.asarray(Wk, dtype=np.float32)
    Wv = np.asarray(Wv, dtype=np.float32)

    nc = _get_nc()

    wqk = np.ascontiguousarray(
        np.concatenate([Wq, Wk], axis=1).reshape(ND, 128, 128)
    ).astype(bf16)
    wv = np.ascontiguousarray(Wv.reshape(ND, 128, HD)).astype(bf16)
    mneg = np.where(
        np.arange(128)[None, :] < np.arange(128)[:, None], -1e5, 0.0
    ).astype(np.float32).astype(bf16)
    identb = np.eye(128, dtype=np.float32).astype(bf16)
    onescol = np.ones((128, NJ), dtype=np.float32).astype(bf16)
    ident = np.eye(64, dtype=np.float32)

    in_maps = []
    for c in range(NCORES):
        xt = np.ascontiguousarray(
            x[BPC * c:BPC * (c + 1)].transpose(0, 2, 1)
        ).astype(bf16)
        in_maps.append(
            {
                "xt": xt,
                "wqk": wqk,
                "wv": wv,
                "mneg": mneg,
                "identb": identb,
                "onescol": onescol,
                "ident": ident,
            }
        )

    kwargs = dict(_trace_kwargs or {})
    res = run_bass_kernel_spmd(
        nc, in_maps, list(range(NCORES)), trace=_trace, **kwargs
    )

    out = np.empty((B, T, HD), dtype=np.float32)
    for c in range(NCORES):
        ot = res.results[c]["ot"]  # [BPC, HD, T]
        out[BPC * c:BPC * (c + 1)] = ot.transpose(0, 2, 1)
    if _trace:
        _cache["last_results"] = res
    return out


# revision 19
# speedup vs baseline: 1.1418x; 1.1396x over previous
"""Causal single-head attention on 8 Trainium2 NeuronCores.

Reference computation (per batch b of 16):
    q = x @ Wq; k = x @ Wk; v = x @ Wv        # x [2048, 512], W* [512, 64]
    out = softmax_causal(q @ k.T / 8) @ v     # out [2048, 64]

Sharding: data-parallel over batch, 2 batches per core, weights replicated.

Per-core kernel (batch-local b in {0,1}), all matmul operands bf16
(psum accumulation fp32; rel-err budget 2e-2 leaves ample margin):
  - host supplies xT = x[b].T in bf16 so the D-contraction sits on
    partitions; xt DMAs issue from the gpsimd queue (parallel to the
    const DMAs on sync) with the Q0 slices first, so the first
    projection starts ~15us earlier than a single-queue issue.
  - qT/kT: psum[0:64]=qT, psum[64:128]=kT via packed lhsT [Wq|Wk]
  - qklo tile = swapped halves of qk (kT at 0:64, qT at 64:128) via
    two SBUF->SBUF DMAs; gives both partition homes needed below.
  - scores TRANSPOSED ST[k, q] so softmax's denominator is a
    partition-dim sum the PV matmul computes via a ones column.
  - ST contraction is only K=64, so even/odd k-blocks run CONCURRENTLY
    in the PE array via 2-way row tiling (tile_position (0,0)/(64,0)):
      even j: lhsT=qklo[0:64](kT),  moving qk[0:64](qT)
      odd  j: lhsT=qk[64:128](kT),  moving qklo[64:128](qT)
    ~2x on the score matmuls.
  - vT via Wv-stationary matmuls; v natural layout produced by the
    DMA-transpose XBAR (16x128 bf16 tiles) straight into v1[:, j, 0:64]
    (no PE transposes); v1[:, :, 64] = 1 makes PV emit the denominator.
  - exp on ACT out of psum in [128, 1024] chunks (pair of k-blocks),
    junk prefix before the first computed column sliced off; output
    bf16 feeds PV directly.
  - causal: k-blocks above the diagonal skipped; diagonal blocks get a
    triangular mask multiply (on the otherwise-idle gpsimd engine) and
    suffix-sliced matmuls.
  - oT[65, 512] accumulates [v|1].T @ p~ over k-blocks in psum; row 64
    is the denominator l; out = oT[0:64] * bcast(1/l) via DVE
    reciprocal_approx_fast (reading psum directly) + gpsimd
    partition_broadcast.
  - projection matmuls of chunk Q+1 are interleaved between attention
    pairs of chunk Q in emission order, so the in-order PE stream fills
    the ACT-gated stalls of the attention inner loop.
  - output written transposed [2, 64, 2048] fp32; host transposes back.
"""

import sys

sys.path.insert(0, "/opt/trn_rl_repo")

import numpy as np

B, T, D, HD = 16, 2048, 512, 64
NCORES = 8
BPC = B // NCORES          # batches per core
NQ = T // 512              # 512-wide q chunks per batch
NJ = T // 128              # 128-wide k blocks per batch
ND = D // 128              # 128-deep contraction tiles

_cache = {}


def _build_nc():
    import concourse.bacc as bacc
    import concourse.mybir as mybir
    import concourse.tile as tile

    F32 = mybir.dt.float32
    BF16 = mybir.dt.bfloat16
    AF = mybir.ActivationFunctionType

    nc = bacc.Bacc("TRN2", target_bir_lowering=False, debug=False)

    xt_d = nc.dram_tensor("xt", [BPC, D, T], BF16, kind="ExternalInput")
    wqk_d = nc.dram_tensor("wqk", [ND, 128, 128], BF16, kind="ExternalInput")
    wv_d = nc.dram_tensor("wv", [ND, 128, HD], BF16, kind="ExternalInput")
    mneg_d = nc.dram_tensor("mneg", [128, 128], BF16, kind="ExternalInput")
    identb_d = nc.dram_tensor("identb", [128, 128], BF16, kind="ExternalInput")
    ident_d = nc.dram_tensor("ident", [64, 64], F32, kind="ExternalInput")
    onescol_d = nc.dram_tensor("onescol", [128, NJ], BF16, kind="ExternalInput")
    ot_d = nc.dram_tensor("ot", [BPC, HD, T], F32, kind="ExternalOutput")

    with tile.TileContext(nc) as tc:
        with (
            tc.tile_pool(name="const", bufs=1) as cpool,
            tc.tile_pool(name="xt", bufs=1) as xtpool,
            tc.tile_pool(name="qk", bufs=2) as qkpool,
            tc.tile_pool(name="qklo", bufs=2) as qklopool,
            tc.tile_pool(name="vt", bufs=2) as vtpool,
            tc.tile_pool(name="v1", bufs=2) as v1pool,
            tc.tile_pool(name="pt", bufs=4) as ptpool,
            tc.tile_pool(name="rec", bufs=2) as recpool,
            tc.tile_pool(name="outp", bufs=2) as outpool,
            tc.tile_pool(name="st", bufs=2, space="PSUM") as stpool,
            tc.tile_pool(name="otp", bufs=2, space="PSUM") as otpool,
            tc.tile_pool(name="aux", bufs=2, space="PSUM") as auxpool,
        ):
            # ---- constants / weights: issued on the scalar (ACT) HWDGE
            # queue so the sync queue's first entries are the qklo shifts
            # the first attention chunk waits on; wqk/wv land as single
            # combined DMAs (5 configs instead of 11 at ~610ns each) ----
            wqk_all = cpool.tile([128, ND, 128], BF16, tag="wqk")
            nc.scalar.dma_start(wqk_all[:], wqk_d[:].rearrange("d p c -> p d c"))
            wqk = [wqk_all[:, d, :] for d in range(ND)]
            wv_all = cpool.tile([128, ND, HD], BF16, tag="wv")
            nc.scalar.dma_start(wv_all[:], wv_d[:].rearrange("d p c -> p d c"))
            wv = [wv_all[:, d, :] for d in range(ND)]
            ident = cpool.tile([64, 64], F32, tag="ident")
            nc.scalar.dma_start(ident[:], ident_d[:])
            mneg = cpool.tile([128, 128], BF16, tag="mneg")
            nc.scalar.dma_start(mneg[:], mneg_d[:])
            identb = cpool.tile([128, 128], BF16, tag="identb")
            nc.scalar.dma_start(identb[:], identb_d[:])
            onescol = cpool.tile([128, NJ], BF16, tag="onescol")
            nc.scalar.dma_start(onescol[:], onescol_d[:])

            # warm the exp table set on ACT while the first loads run
            scratch = cpool.tile([1, 1], F32, tag="scratch")
            nc.scalar.activation(scratch[:], scratch[:], AF.Exp)

            # PE clock warm-up: dependency-free junk matmuls ramp the
            # tensor engine out of its low p-state (0.65 -> 2.4 GHz needs
            # ~3us of continuous execution) while the x DMAs are in flight,
            # so proj(0,0) runs at full clock instead of 3.7x slower.
            wscr = cpool.tile([128, 512], BF16, tag="wscr")
            nc.gpsimd.memset(wscr[:], 0.5)
            for i in range(6):
                pw = auxpool.tile([128, 512], F32, tag="aux", name="pwarm")
                nc.tensor.matmul(
                    pw[:], wscr[:, 0:128], wscr[:], start=True, stop=True
                )

            # ---- x loads (gpsimd queue, Q0 slices first) ----
            xts = {}
            for b in range(BPC):
                for d in range(ND):
                    xts[(b, d)] = xtpool.tile(
                        [128, T], BF16, tag=f"xt{b}{d}", name=f"xt{b}{d}"
                    )
            for d in range(ND):
                nc.gpsimd.dma_start(
                    xts[(0, d)][:, 0:512], xt_d[0, 128 * d:128 * (d + 1), 0:512]
                )
            for d in range(ND):
                nc.gpsimd.dma_start(
                    xts[(0, d)][:, 512:T], xt_d[0, 128 * d:128 * (d + 1), 512:T]
                )
            def emit_b1_xt():
                for d in range(ND):
                    nc.gpsimd.dma_start(
                        xts[(1, d)][:, :], xt_d[1, 128 * d:128 * (d + 1), :]
                    )

            qks, qklos, v1s, vts = {}, {}, {}, {}
            for b in range(BPC):
                qks[b] = qkpool.tile([128, T], BF16, tag="qk", name=f"qk{b}")
                qklos[b] = qklopool.tile([128, T], BF16, tag="qklo", name=f"qklo{b}")
                v1s[b] = v1pool.tile([128, NJ, HD + 1], BF16, tag="v1", name=f"v1{b}")
                vts[b] = vtpool.tile([64, T], F32, tag="vt", name=f"vt{b}")
            for b in range(BPC):
                nc.vector.tensor_copy(
                    v1s[b][:, :, HD:HD + 1],
                    onescol[:].rearrange("p (a c) -> p a c", c=1),
                )

            def proj_steps(b, Q):
                """Emission steps for everything attention chunk (b, Q) needs
                from tokens [512Q, 512Q+512): returns a list of callables so
                the caller can interleave them between attention pairs."""
                s = slice(512 * Q, 512 * (Q + 1))
                qk, qklo, v1, vt = qks[b], qklos[b], v1s[b], vts[b]
                state = {}

                def mk_qk(d):
                    def f():
                        if d == 0:
                            state["pqk"] = auxpool.tile(
                                [128, 512], F32, tag="aux", name="pqk"
                            )
                        nc.tensor.matmul(
                            state["pqk"][:], wqk[d], xts[(b, d)][:, s],
                            start=(d == 0), stop=(d == ND - 1),
                        )
                        if d == ND - 1:
                            nc.vector.tensor_copy(qk[:, s], state["pqk"][:])
                            # swapped halves: kT to 0:64, qT to 64:128
                            nc.sync.dma_start(qklo[0:64, s], qk[64:128, s])
                            nc.scalar.dma_start(qklo[64:128, s], qk[0:64, s])
                    return f

                def mk_v(d):
                    def f():
                        if d == 0:
                            state["pv"] = auxpool.tile(
                                [64, 512], F32, tag="aux", name="pv"
                            )
                        nc.tensor.matmul(
                            state["pv"][:], wv[d], xts[(b, d)][:, s],
                            start=(d == 0), stop=(d == ND - 1),
                        )
                        if d == ND - 1:
                            nc.vector.tensor_copy(vt[:, s], state["pv"][:])
                            for t2 in range(2 * Q, 2 * Q + 2):
                                p2 = auxpool.tile(
                                    [128, 128], F32, tag="aux", name="ptr"
                                )
                                for tt in range(2):
                                    nc.tensor.transpose(
                                        p2[:, 64 * tt:64 * (tt + 1)],
                                        vt[:, 128 * (2 * t2 + tt):
                                           128 * (2 * t2 + tt + 1)],
                                        ident[:],
                                    )
                                nc.vector.tensor_copy(
                                    v1[:, 2 * t2:2 * t2 + 2, 0:HD],
                                    p2[:].rearrange("p (a c) -> p a c", a=2),
                                )
                    return f

                return [mk_qk(d) for d in range(ND)] + [mk_v(d) for d in range(ND)]

            def emit_attn_q(b, Q, fillers):
                """One query chunk: all causal k-blocks in even/odd pairs
                run concurrently via 2-way PE row tiling; PV skewed one
                pair behind ST; proj steps for the next chunk interleaved."""
                qk, qklo, v1 = qks[b], qklos[b], v1s[b]
                pot = otpool.tile([HD + 1, 512], F32, tag="ot", name="pot")
                njb = 4 * (Q + 1)
                jlast = njb - 1

                def emit_pv(p_tile, g):
                    je, jo = 2 * g, 2 * g + 1
                    w0e = 128 * (je - 4 * Q) if je >= 4 * Q else 0
                    w0o = 128 * (jo - 4 * Q) if jo >= 4 * Q else 0
                    nc.tensor.matmul(
                        pot[:, w0e:512], v1[:, je, :], p_tile[:, w0e:512],
                        start=(je == 0), stop=False,
                    )
                    nc.tensor.matmul(
                        pot[:, w0o:512], v1[:, jo, :],
                        p_tile[:, 512 + w0o:1024],
                        start=False, stop=(jo == jlast),
                    )

                pending = []
                for g in range(njb // 2):
                    je, jo = 2 * g, 2 * g + 1
                    w0e = 128 * (je - 4 * Q) if je >= 4 * Q else 0
                    w0o = 128 * (jo - 4 * Q) if jo >= 4 * Q else 0
                    diag_e, diag_o = je >= 4 * Q, jo >= 4 * Q
                    pst = stpool.tile([128, 1024], F32, tag="st", name="pst")
                    # even block: kT stationary on partitions 0:64 (rg 0)
                    nc.tensor.matmul(
                        pst[:, w0e:512],
                        qklo[0:64, 128 * je:128 * (je + 1)],
                        qk[0:64, 512 * Q + w0e:512 * (Q + 1)],
                        start=True, stop=not diag_e,
                    )
                    # odd block: kT stationary on partitions 64:128 (rg 64)
                    nc.tensor.matmul(
                        pst[:, 512 + w0o:1024],
                        qk[64:128, 128 * jo:128 * (jo + 1)],
                        qklo[64:128, 512 * Q + w0o:512 * (Q + 1)],
                        start=True, stop=not diag_o,
                    )
                    # diagonal blocks: add -1e5 above the diagonal ON the PE
                    # (I.T @ mneg accumulated into the 128-wide diag window),
                    # so exp underflows to exactly 0 and no cross-engine mask
                    # op sits between exp and PV
                    if diag_e:
                        nc.tensor.matmul(
                            pst[:, w0e:w0e + 128], identb[:], mneg[:],
                            start=False, stop=True,
                        )
                    if diag_o:
                        nc.tensor.matmul(
                            pst[:, 512 + w0o:512 + w0o + 128], identb[:],
                            mneg[:], start=False, stop=True,
                        )
                    if fillers:
                        fillers.pop(0)()
                    ptil = ptpool.tile([128, 1024], BF16, tag="pt", name="ptil")
                    nc.scalar.activation(
                        ptil[:, w0e:1024], pst[:, w0e:1024], AF.Exp,
                        scale=1.0 / np.sqrt(HD),
                    )
                    # PV runs two pairs behind ST so the in-order PE stream
                    # never waits on the exp of the pair it just issued
                    pending.append((ptil, g))
                    if len(pending) > 2:
                        emit_pv(*pending.pop(0))
                for p in pending:
                    if fillers:
                        fillers.pop(0)()
                    emit_pv(*p)
                while fillers:
                    fillers.pop(0)()

                # normalize: out = oT[0:64] * broadcast(1 / l), PE-free
                # (reciprocal_approx_fast is a custom-DVE op that mishandles
                #  psum APs at a nonzero base partition — stage l via SBUF)
                lsb = recpool.tile([1, 512], F32, tag="lsb", name="lsb")
                nc.vector.tensor_copy(lsb[:], pot[HD:HD + 1, :])
                rsb = recpool.tile([1, 512], F32, tag="rsb", name="rsb")
                nc.vector.reciprocal_approx_fast(rsb[:], lsb[:])
                rcb = outpool.tile([HD, 512], F32, tag="rcb", name="rcb")
                nc.gpsimd.partition_broadcast(rcb[:], rsb[:])
                osb = outpool.tile([HD, 512], F32, tag="out", name="osb")
                nc.vector.tensor_mul(osb[:], pot[0:HD, :], rcb[:])
                nc.sync.dma_start(
                    ot_d[b, :, 512 * Q:512 * (Q + 1)], osb[:]
                )

            # ---- emission schedule (no fillers bisect) ----
            import os
            if os.environ.get("KNOFILL"):
                for b in range(BPC):
                    if b == 1:
                        emit_b1_xt()
                    for Q in range(NQ):
                        for f in proj_steps(b, Q):
                            f()
                        emit_attn_q(b, Q, [])
            else:
                for f in proj_steps(0, 0):
                    f()
                emit_attn_q(0, 0, proj_steps(0, 1))
                emit_attn_q(0, 1, proj_steps(0, 2))
                emit_b1_xt()
                emit_attn_q(0, 2, proj_steps(0, 3))
                emit_attn_q(0, 3, proj_steps(1, 0))
                emit_attn_q(1, 0, proj_steps(1, 1))
                emit_attn_q(1, 1, proj_steps(1, 2))
                emit_attn_q(1, 2, proj_steps(1, 3))
                emit_attn_q(1, 3, [])

    nc.compile()
    return nc


def _get_nc():
    if "nc" not in _cache:
        _cache["nc"] = _build_nc()
    return _cache["nc"]


def kernel(x, Wq, Wk, Wv, _trace=False, _trace_kwargs=None):
    import ml_dtypes
    from concourse.bass_utils import run_bass_kernel_spmd

    bf16 = ml_dtypes.bfloat16
    x = np.asarray(x, dtype=np.float32)
    Wq = np.asarray(Wq, dtype=np.float32)
    Wk = np.asarray(Wk, dtype=np.float32)
    Wv = np.asarray(Wv, dtype=np.float32)

    nc = _get_nc()

    wqk = np.ascontiguousarray(
        np.concatenate([Wq, Wk], axis=1).reshape(ND, 128, 128)
    ).astype(bf16)
    wv = np.ascontiguousarray(Wv.reshape(ND, 128, HD)).astype(bf16)
    mneg = np.where(
        np.arange(128)[None, :] < np.arange(128)[:, None], -1e5, 0.0
    ).astype(np.float32).astype(bf16)
    identb = np.eye(128, dtype=np.float32).astype(bf16)
    onescol = np.ones((128, NJ), dtype=np.float32).astype(bf16)
    ident = np.eye(64, dtype=np.float32)

    in_maps = []
    for c in range(NCORES):
        xt = np.ascontiguousarray(
            x[BPC * c:BPC * (c + 1)].transpose(0, 2, 1)
        ).astype(bf16)
        in_maps.append(
            {
                "xt": xt,
                "wqk": wqk,
                "wv": wv,
                "mneg": mneg,
                "identb": identb,
                "onescol": onescol,
                "ident": ident,
            }
        )

    kwargs = dict(_trace_kwargs or {})
    res = run_bass_kernel_spmd(
        nc, in_maps, list(range(NCORES)), trace=_trace, **kwargs
    )

    out = np.empty((B, T, HD), dtype=np.float32)
    for c in range(NCORES):
        ot = res.results[c]["ot"]  # [BPC, HD, T]
        out[BPC * c:BPC * (c + 1)] = ot.transpose(0, 2, 1)
    if _trace:
        _cache["last_results"] = res
    return out


# revision 20
# speedup vs baseline: 1.1554x; 1.0119x over previous
"""Causal single-head attention on 8 Trainium2 NeuronCores.

Reference computation (per batch b of 16):
    q = x @ Wq; k = x @ Wk; v = x @ Wv        # x [2048, 512], W* [512, 64]
    out = softmax_causal(q @ k.T / 8) @ v     # out [2048, 64]

Sharding: data-parallel over batch, 2 batches per core, weights replicated.

Per-core kernel (batch-local b in {0,1}), all matmul operands bf16
(psum accumulation fp32; rel-err budget 2e-2 leaves ample margin):
  - host supplies xT = x[b].T in bf16 so the D-contraction sits on
    partitions; xt DMAs issue from the gpsimd queue (parallel to the
    const DMAs on sync) with the Q0 slices first, so the first
    projection starts ~15us earlier than a single-queue issue.
  - qT/kT: psum[0:64]=qT, psum[64:128]=kT via packed lhsT [Wq|Wk]
  - qklo tile = swapped halves of qk (kT at 0:64, qT at 64:128) via
    two SBUF->SBUF DMAs; gives both partition homes needed below.
  - scores TRANSPOSED ST[k, q] so softmax's denominator is a
    partition-dim sum the PV matmul computes via a ones column.
  - ST contraction is only K=64, so even/odd k-blocks run CONCURRENTLY
    in the PE array via 2-way row tiling (tile_position (0,0)/(64,0)):
      even j: lhsT=qklo[0:64](kT),  moving qk[0:64](qT)
      odd  j: lhsT=qk[64:128](kT),  moving qklo[64:128](qT)
    ~2x on the score matmuls.
  - vT via Wv-stationary matmuls; v natural layout produced by the
    DMA-transpose XBAR (16x128 bf16 tiles) straight into v1[:, j, 0:64]
    (no PE transposes); v1[:, :, 64] = 1 makes PV emit the denominator.
  - exp on ACT out of psum in [128, 1024] chunks (pair of k-blocks),
    junk prefix before the first computed column sliced off; output
    bf16 feeds PV directly.
  - causal: k-blocks above the diagonal skipped; diagonal blocks get a
    triangular mask multiply (on the otherwise-idle gpsimd engine) and
    suffix-sliced matmuls.
  - oT[65, 512] accumulates [v|1].T @ p~ over k-blocks in psum; row 64
    is the denominator l; out = oT[0:64] * bcast(1/l) via DVE
    reciprocal_approx_fast (reading psum directly) + gpsimd
    partition_broadcast.
  - projection matmuls of chunk Q+1 are interleaved between attention
    pairs of chunk Q in emission order, so the in-order PE stream fills
    the ACT-gated stalls of the attention inner loop.
  - output written transposed [2, 64, 2048] fp32; host transposes back.
"""

import sys

sys.path.insert(0, "/opt/trn_rl_repo")

import numpy as np

B, T, D, HD = 16, 2048, 512, 64
NCORES = 8
BPC = B // NCORES          # batches per core
NQ = T // 512              # 512-wide q chunks per batch
NJ = T // 128              # 128-wide k blocks per batch
ND = D // 128              # 128-deep contraction tiles

_cache = {}


def _build_nc():
    import concourse.bacc as bacc
    import concourse.mybir as mybir
    import concourse.tile as tile

    F32 = mybir.dt.float32
    BF16 = mybir.dt.bfloat16
    AF = mybir.ActivationFunctionType

    nc = bacc.Bacc("TRN2", target_bir_lowering=False, debug=False)

    xt_d = nc.dram_tensor("xt", [BPC, D, T], BF16, kind="ExternalInput")
    wqk_d = nc.dram_tensor("wqk", [ND, 128, 128], BF16, kind="ExternalInput")
    wv_d = nc.dram_tensor("wv", [ND, 128, HD], BF16, kind="ExternalInput")
    mneg_d = nc.dram_tensor("mneg", [128, 128], BF16, kind="ExternalInput")
    identb_d = nc.dram_tensor("identb", [128, 128], BF16, kind="ExternalInput")
    ident_d = nc.dram_tensor("ident", [64, 64], F32, kind="ExternalInput")
    onescol_d = nc.dram_tensor("onescol", [128, NJ], BF16, kind="ExternalInput")
    ot_d = nc.dram_tensor("ot", [BPC, HD, T], F32, kind="ExternalOutput")

    with tile.TileContext(nc) as tc:
        with (
            tc.tile_pool(name="const", bufs=1) as cpool,
            tc.tile_pool(name="xt", bufs=1) as xtpool,
            tc.tile_pool(name="qk", bufs=2) as qkpool,
            tc.tile_pool(name="qklo", bufs=2) as qklopool,
            tc.tile_pool(name="vt", bufs=2) as vtpool,
            tc.tile_pool(name="v1", bufs=2) as v1pool,
            tc.tile_pool(name="pt", bufs=4) as ptpool,
            tc.tile_pool(name="rec", bufs=2) as recpool,
            tc.tile_pool(name="outp", bufs=2) as outpool,
            tc.tile_pool(name="st", bufs=2, space="PSUM") as stpool,
            tc.tile_pool(name="otp", bufs=2, space="PSUM") as otpool,
            tc.tile_pool(name="aux", bufs=2, space="PSUM") as auxpool,
        ):
            # ---- constants / weights: issued on the scalar (ACT) HWDGE
            # queue so the sync queue's first entries are the qklo shifts
            # the first attention chunk waits on; wqk/wv land as single
            # combined DMAs (5 configs instead of 11 at ~610ns each) ----
            wqk_all = cpool.tile([128, ND, 128], BF16, tag="wqk")
            nc.scalar.dma_start(wqk_all[:], wqk_d[:].rearrange("d p c -> p d c"))
            wqk = [wqk_all[:, d, :] for d in range(ND)]
            wv_all = cpool.tile([128, ND, HD], BF16, tag="wv")
            nc.scalar.dma_start(wv_all[:], wv_d[:].rearrange("d p c -> p d c"))
            wv = [wv_all[:, d, :] for d in range(ND)]
            ident = cpool.tile([64, 64], F32, tag="ident")
            nc.scalar.dma_start(ident[:], ident_d[:])
            mneg = cpool.tile([128, 128], BF16, tag="mneg")
            nc.scalar.dma_start(mneg[:], mneg_d[:])
            identb = cpool.tile([128, 128], BF16, tag="identb")
            nc.scalar.dma_start(identb[:], identb_d[:])
            onescol = cpool.tile([128, NJ], BF16, tag="onescol")
            nc.scalar.dma_start(onescol[:], onescol_d[:])

            # warm the exp table set on ACT while the first loads run
            scratch = cpool.tile([1, 1], F32, tag="scratch")
            nc.scalar.activation(scratch[:], scratch[:], AF.Exp)

            # PE clock warm-up: dependency-free junk matmuls ramp the
            # tensor engine out of its low p-state (0.65 -> 2.4 GHz needs
            # ~3us of continuous execution) while the x DMAs are in flight,
            # so proj(0,0) runs at full clock instead of 3.7x slower.
            wscr = cpool.tile([128, 512], BF16, tag="wscr")
            nc.gpsimd.memset(wscr[:], 0.5)
            for i in range(10):
                pw = auxpool.tile([128, 512], F32, tag="aux", name="pwarm")
                nc.tensor.matmul(
                    pw[:], wscr[:, 0:128], wscr[:], start=True, stop=True
                )

            # ---- x loads (gpsimd queue, Q0 slices first) ----
            xts = {}
            for b in range(BPC):
                for d in range(ND):
                    xts[(b, d)] = xtpool.tile(
                        [128, T], BF16, tag=f"xt{b}{d}", name=f"xt{b}{d}"
                    )
            for d in range(ND):
                nc.gpsimd.dma_start(
                    xts[(0, d)][:, 0:512], xt_d[0, 128 * d:128 * (d + 1), 0:512]
                )
            for d in range(ND):
                nc.gpsimd.dma_start(
                    xts[(0, d)][:, 512:T], xt_d[0, 128 * d:128 * (d + 1), 512:T]
                )
            def emit_b1_xt():
                for d in range(ND):
                    nc.gpsimd.dma_start(
                        xts[(1, d)][:, :], xt_d[1, 128 * d:128 * (d + 1), :]
                    )

            qks, qklos, v1s, vts = {}, {}, {}, {}
            for b in range(BPC):
                qks[b] = qkpool.tile([128, T], BF16, tag="qk", name=f"qk{b}")
                qklos[b] = qklopool.tile([128, T], BF16, tag="qklo", name=f"qklo{b}")
                v1s[b] = v1pool.tile([128, NJ, HD + 1], BF16, tag="v1", name=f"v1{b}")
                vts[b] = vtpool.tile([64, T], F32, tag="vt", name=f"vt{b}")
            for b in range(BPC):
                nc.vector.tensor_copy(
                    v1s[b][:, :, HD:HD + 1],
                    onescol[:].rearrange("p (a c) -> p a c", c=1),
                )

            def proj_steps(b, Q):
                """Emission steps for everything attention chunk (b, Q) needs
                from tokens [512Q, 512Q+512): returns a list of callables so
                the caller can interleave them between attention pairs."""
                s = slice(512 * Q, 512 * (Q + 1))
                qk, qklo, v1, vt = qks[b], qklos[b], v1s[b], vts[b]
                state = {}

                def mk_qk(d):
                    def f():
                        if d == 0:
                            state["pqk"] = auxpool.tile(
                                [128, 512], F32, tag="aux", name="pqk"
                            )
                        nc.tensor.matmul(
                            state["pqk"][:], wqk[d], xts[(b, d)][:, s],
                            start=(d == 0), stop=(d == ND - 1),
                        )
                        if d == ND - 1:
                            nc.vector.tensor_copy(qk[:, s], state["pqk"][:])
                            # swapped halves: kT to 0:64, qT to 64:128
                            nc.sync.dma_start(qklo[0:64, s], qk[64:128, s])
                            nc.scalar.dma_start(qklo[64:128, s], qk[0:64, s])
                    return f

                def mk_v(d):
                    def f():
                        if d == 0:
                            state["pv"] = auxpool.tile(
                                [64, 512], F32, tag="aux", name="pv"
                            )
                        nc.tensor.matmul(
                            state["pv"][:], wv[d], xts[(b, d)][:, s],
                            start=(d == 0), stop=(d == ND - 1),
                        )
                        if d == ND - 1:
                            nc.vector.tensor_copy(vt[:, s], state["pv"][:])
                            for t2 in range(2 * Q, 2 * Q + 2):
                                p2 = auxpool.tile(
                                    [128, 128], F32, tag="aux", name="ptr"
                                )
                                for tt in range(2):
                                    nc.tensor.transpose(
                                        p2[:, 64 * tt:64 * (tt + 1)],
                                        vt[:, 128 * (2 * t2 + tt):
                                           128 * (2 * t2 + tt + 1)],
                                        ident[:],
                                    )
                                nc.vector.tensor_copy(
                                    v1[:, 2 * t2:2 * t2 + 2, 0:HD],
                                    p2[:].rearrange("p (a c) -> p a c", a=2),
                                )
                    return f

                return [mk_qk(d) for d in range(ND)] + [mk_v(d) for d in range(ND)]

            def emit_attn_q(b, Q, fillers):
                """One query chunk: all causal k-blocks in even/odd pairs
                run concurrently via 2-way PE row tiling; PV skewed one
                pair behind ST; proj steps for the next chunk interleaved."""
                qk, qklo, v1 = qks[b], qklos[b], v1s[b]
                pot = otpool.tile([HD + 1, 512], F32, tag="ot", name="pot")
                njb = 4 * (Q + 1)
                jlast = njb - 1

                def emit_pv(p_tile, g):
                    je, jo = 2 * g, 2 * g + 1
                    w0e = 128 * (je - 4 * Q) if je >= 4 * Q else 0
                    w0o = 128 * (jo - 4 * Q) if jo >= 4 * Q else 0
                    nc.tensor.matmul(
                        pot[:, w0e:512], v1[:, je, :], p_tile[:, w0e:512],
                        start=(je == 0), stop=False,
                    )
                    nc.tensor.matmul(
                        pot[:, w0o:512], v1[:, jo, :],
                        p_tile[:, 512 + w0o:1024],
                        start=False, stop=(jo == jlast),
                    )

                pending = []
                for g in range(njb // 2):
                    je, jo = 2 * g, 2 * g + 1
                    w0e = 128 * (je - 4 * Q) if je >= 4 * Q else 0
                    w0o = 128 * (jo - 4 * Q) if jo >= 4 * Q else 0
                    diag_e, diag_o = je >= 4 * Q, jo >= 4 * Q
                    pst = stpool.tile([128, 1024], F32, tag="st", name="pst")
                    # even block: kT stationary on partitions 0:64 (rg 0)
                    nc.tensor.matmul(
                        pst[:, w0e:512],
                        qklo[0:64, 128 * je:128 * (je + 1)],
                        qk[0:64, 512 * Q + w0e:512 * (Q + 1)],
                        start=True, stop=not diag_e,
                    )
                    # odd block: kT stationary on partitions 64:128 (rg 64)
                    nc.tensor.matmul(
                        pst[:, 512 + w0o:1024],
                        qk[64:128, 128 * jo:128 * (jo + 1)],
                        qklo[64:128, 512 * Q + w0o:512 * (Q + 1)],
                        start=True, stop=not diag_o,
                    )
                    # diagonal blocks: add -1e5 above the diagonal ON the PE
                    # (I.T @ mneg accumulated into the 128-wide diag window),
                    # so exp underflows to exactly 0 and no cross-engine mask
                    # op sits between exp and PV
                    if diag_e:
                        nc.tensor.matmul(
                            pst[:, w0e:w0e + 128], identb[:], mneg[:],
                            start=False, stop=True,
                        )
                    if diag_o:
                        nc.tensor.matmul(
                            pst[:, 512 + w0o:512 + w0o + 128], identb[:],
                            mneg[:], start=False, stop=True,
                        )
                    if fillers:
                        fillers.pop(0)()
                    ptil = ptpool.tile([128, 1024], BF16, tag="pt", name="ptil")
                    nc.scalar.activation(
                        ptil[:, w0e:1024], pst[:, w0e:1024], AF.Exp,
                        scale=1.0 / np.sqrt(HD),
                    )
                    # PV runs two pairs behind ST so the in-order PE stream
                    # never waits on the exp of the pair it just issued
                    pending.append((ptil, g))
                    if len(pending) > 2:
                        emit_pv(*pending.pop(0))
                for p in pending:
                    if fillers:
                        fillers.pop(0)()
                    emit_pv(*p)
                while fillers:
                    fillers.pop(0)()

                # normalize: out = oT[0:64] * broadcast(1 / l), PE-free
                # (reciprocal_approx_fast is a custom-DVE op that mishandles
                #  psum APs at a nonzero base partition — stage l via SBUF)
                lsb = recpool.tile([1, 512], F32, tag="lsb", name="lsb")
                nc.vector.tensor_copy(lsb[:], pot[HD:HD + 1, :])
                rsb = recpool.tile([1, 512], F32, tag="rsb", name="rsb")
                nc.vector.reciprocal_approx_fast(rsb[:], lsb[:])
                rcb = outpool.tile([HD, 512], F32, tag="rcb", name="rcb")
                nc.gpsimd.partition_broadcast(rcb[:], rsb[:])
                osb = outpool.tile([HD, 512], F32, tag="out", name="osb")
                nc.vector.tensor_mul(osb[:], pot[0:HD, :], rcb[:])
                nc.sync.dma_start(
                    ot_d[b, :, 512 * Q:512 * (Q + 1)], osb[:]
                )

            # ---- emission schedule (no fillers bisect) ----
            import os
            if os.environ.get("KNOFILL"):
                for b in range(BPC):
                    if b == 1:
                        emit_b1_xt()
                    for Q in range(NQ):
                        for f in proj_steps(b, Q):
                            f()
                        emit_attn_q(b, Q, [])
            else:
                for f in proj_steps(0, 0):
                    f()
                emit_attn_q(0, 0, proj_steps(0, 1))
                emit_attn_q(0, 1, proj_steps(0, 2))
                emit_b1_xt()
                emit_attn_q(0, 2, proj_steps(0, 3))
                emit_attn_q(0, 3, proj_steps(1, 0))
                emit_attn_q(1, 0, proj_steps(1, 1))
                emit_attn_q(1, 1, proj_steps(1, 2))
                emit_attn_q(1, 2, proj_steps(1, 3))
                emit_attn_q(1, 3, [])

    nc.compile()
    return nc


def _get_nc():
    if "nc" not in _cache:
        _cache["nc"] = _build_nc()
    return _cache["nc"]


def kernel(x, Wq, Wk, Wv, _trace=False, _trace_kwargs=None):
    import ml_dtypes
    from concourse.bass_utils import run_bass_kernel_spmd

    bf16 = ml_dtypes.bfloat16
    x = np.asarray(x, dtype=np.float32)
    Wq = np.asarray(Wq, dtype=np.float32)
    Wk = np.asarray(Wk, dtype=np.float32)
    Wv = np.asarray(Wv, dtype=np.float32)

    nc = _get_nc()

    wqk = np.ascontiguousarray(
        np.concatenate([Wq, Wk], axis=1).reshape(ND, 128, 128)
    ).astype(bf16)
    wv = np.ascontiguousarray(Wv.reshape(ND, 128, HD)).astype(bf16)
    mneg = np.where(
        np.arange(128)[None, :] < np.arange(128)[:, None], -1e5, 0.0
    ).astype(np.float32).astype(bf16)
    identb = np.eye(128, dtype=np.float32).astype(bf16)
    onescol = np.ones((128, NJ), dtype=np.float32).astype(bf16)
    ident = np.eye(64, dtype=np.float32)

    in_maps = []
    for c in range(NCORES):
        xt = np.ascontiguousarray(
            x[BPC * c:BPC * (c + 1)].transpose(0, 2, 1)
        ).astype(bf16)
        in_maps.append(
            {
                "xt": xt,
                "wqk": wqk,
                "wv": wv,
                "mneg": mneg,
                "identb": identb,
                "onescol": onescol,
                "ident": ident,
            }
        )

    kwargs = dict(_trace_kwargs or {})
    res = run_bass_kernel_spmd(
        nc, in_maps, list(range(NCORES)), trace=_trace, **kwargs
    )

    out = np.empty((B, T, HD), dtype=np.float32)
    for c in range(NCORES):
        ot = res.results[c]["ot"]  # [BPC, HD, T]
        out[BPC * c:BPC * (c + 1)] = ot.transpose(0, 2, 1)
    if _trace:
        _cache["last_results"] = res
    return out


# revision 21
# speedup vs baseline: 1.1742x; 1.0163x over previous
"""Causal single-head attention on 8 Trainium2 NeuronCores.

Reference computation (per batch b of 16):
    q = x @ Wq; k = x @ Wk; v = x @ Wv        # x [2048, 512], W* [512, 64]
    out = softmax_causal(q @ k.T / 8) @ v     # out [2048, 64]

Sharding: data-parallel over batch, 2 batches per core, weights replicated.

Per-core kernel (batch-local b in {0,1}), all matmul operands bf16
(psum accumulation fp32; rel-err budget 2e-2 leaves ample margin):
  - host supplies xT = x[b].T in bf16 so the D-contraction sits on
    partitions; xt DMAs issue from the gpsimd queue (parallel to the
    const DMAs on sync) with the Q0 slices first, so the first
    projection starts ~15us earlier than a single-queue issue.
  - qT/kT: psum[0:64]=qT, psum[64:128]=kT via packed lhsT [Wq|Wk]
  - qklo tile = swapped halves of qk (kT at 0:64, qT at 64:128) via
    two SBUF->SBUF DMAs; gives both partition homes needed below.
  - scores TRANSPOSED ST[k, q] so softmax's denominator is a
    partition-dim sum the PV matmul computes via a ones column.
  - ST contraction is only K=64, so even/odd k-blocks run CONCURRENTLY
    in the PE array via 2-way row tiling (tile_position (0,0)/(64,0)):
      even j: lhsT=qklo[0:64](kT),  moving qk[0:64](qT)
      odd  j: lhsT=qk[64:128](kT),  moving qklo[64:128](qT)
    ~2x on the score matmuls.
  - vT via Wv-stationary matmuls; v natural layout produced by the
    DMA-transpose XBAR (16x128 bf16 tiles) straight into v1[:, j, 0:64]
    (no PE transposes); v1[:, :, 64] = 1 makes PV emit the denominator.
  - exp on ACT out of psum in [128, 1024] chunks (pair of k-blocks),
    junk prefix before the first computed column sliced off; output
    bf16 feeds PV directly.
  - causal: k-blocks above the diagonal skipped; diagonal blocks get a
    triangular mask multiply (on the otherwise-idle gpsimd engine) and
    suffix-sliced matmuls.
  - oT[65, 512] accumulates [v|1].T @ p~ over k-blocks in psum; row 64
    is the denominator l; out = oT[0:64] * bcast(1/l) via DVE
    reciprocal_approx_fast (reading psum directly) + gpsimd
    partition_broadcast.
  - projection matmuls of chunk Q+1 are interleaved between attention
    pairs of chunk Q in emission order, so the in-order PE stream fills
    the ACT-gated stalls of the attention inner loop.
  - output written transposed [2, 64, 2048] fp32; host transposes back.
"""

import sys

sys.path.insert(0, "/opt/trn_rl_repo")

import numpy as np

B, T, D, HD = 16, 2048, 512, 64
NCORES = 8
BPC = B // NCORES          # batches per core
NQ = T // 512              # 512-wide q chunks per batch
NJ = T // 128              # 128-wide k blocks per batch
ND = D // 128              # 128-deep contraction tiles

_cache = {}


def _build_nc():
    import concourse.bacc as bacc
    import concourse.mybir as mybir
    import concourse.tile as tile

    F32 = mybir.dt.float32
    BF16 = mybir.dt.bfloat16
    AF = mybir.ActivationFunctionType

    nc = bacc.Bacc("TRN2", target_bir_lowering=False, debug=False)

    xt_d = nc.dram_tensor("xt", [BPC, D, T], BF16, kind="ExternalInput")
    wqk_d = nc.dram_tensor("wqk", [ND, 128, 128], BF16, kind="ExternalInput")
    wv_d = nc.dram_tensor("wv", [ND, 128, HD], BF16, kind="ExternalInput")
    mneg_d = nc.dram_tensor("mneg", [128, 128], BF16, kind="ExternalInput")
    identb_d = nc.dram_tensor("identb", [128, 128], BF16, kind="ExternalInput")
    ident_d = nc.dram_tensor("ident", [64, 64], F32, kind="ExternalInput")
    onescol_d = nc.dram_tensor("onescol", [128, NJ], BF16, kind="ExternalInput")
    ot_d = nc.dram_tensor("ot", [BPC, HD, T], F32, kind="ExternalOutput")

    with tile.TileContext(nc) as tc:
        with (
            tc.tile_pool(name="const", bufs=1) as cpool,
            tc.tile_pool(name="xt", bufs=1) as xtpool,
            tc.tile_pool(name="qk", bufs=2) as qkpool,
            tc.tile_pool(name="qklo", bufs=2) as qklopool,
            tc.tile_pool(name="vt", bufs=2) as vtpool,
            tc.tile_pool(name="v1", bufs=2) as v1pool,
            tc.tile_pool(name="pt", bufs=4) as ptpool,
            tc.tile_pool(name="rec", bufs=2) as recpool,
            tc.tile_pool(name="outp", bufs=2) as outpool,
            tc.tile_pool(name="st", bufs=2, space="PSUM") as stpool,
            tc.tile_pool(name="otp", bufs=2, space="PSUM") as otpool,
            tc.tile_pool(name="aux", bufs=2, space="PSUM") as auxpool,
        ):
            # ---- constants / weights: issued on the scalar (ACT) HWDGE
            # queue so the sync queue's first entries are the qklo shifts
            # the first attention chunk waits on; wqk/wv land as single
            # combined DMAs (5 configs instead of 11 at ~610ns each) ----
            wqk_all = cpool.tile([128, ND, 128], BF16, tag="wqk")
            nc.scalar.dma_start(wqk_all[:], wqk_d[:].rearrange("d p c -> p d c"))
            wqk = [wqk_all[:, d, :] for d in range(ND)]
            wv_all = cpool.tile([128, ND, HD], BF16, tag="wv")
            nc.scalar.dma_start(wv_all[:], wv_d[:].rearrange("d p c -> p d c"))
            wv = [wv_all[:, d, :] for d in range(ND)]
            ident = cpool.tile([64, 64], F32, tag="ident")
            nc.scalar.dma_start(ident[:], ident_d[:])
            mneg = cpool.tile([128, 128], BF16, tag="mneg")
            nc.scalar.dma_start(mneg[:], mneg_d[:])
            identb = cpool.tile([128, 128], BF16, tag="identb")
            nc.scalar.dma_start(identb[:], identb_d[:])
            onescol = cpool.tile([128, NJ], BF16, tag="onescol")
            nc.scalar.dma_start(onescol[:], onescol_d[:])

            # warm the exp table set on ACT while the first loads run
            scratch = cpool.tile([1, 1], F32, tag="scratch")
            nc.scalar.activation(scratch[:], scratch[:], AF.Exp)

            # PE clock warm-up: dependency-free junk matmuls ramp the
            # tensor engine out of its low p-state (0.65 -> 2.4 GHz needs
            # ~3us of continuous execution) while the x DMAs are in flight,
            # so proj(0,0) runs at full clock instead of 3.7x slower.
            wscr = cpool.tile([128, 512], BF16, tag="wscr")
            nc.gpsimd.memset(wscr[:], 0.5)
            for i in range(25):
                pw = auxpool.tile([128, 512], F32, tag="aux", name="pwarm")
                nc.tensor.matmul(
                    pw[:], wscr[:, 0:128], wscr[:], start=True, stop=True
                )

            # ---- x loads (gpsimd queue, Q0 slices first) ----
            xts = {}
            for b in range(BPC):
                for d in range(ND):
                    xts[(b, d)] = xtpool.tile(
                        [128, T], BF16, tag=f"xt{b}{d}", name=f"xt{b}{d}"
                    )
            for d in range(ND):
                nc.gpsimd.dma_start(
                    xts[(0, d)][:, 0:512], xt_d[0, 128 * d:128 * (d + 1), 0:512]
                )
            for d in range(ND):
                nc.gpsimd.dma_start(
                    xts[(0, d)][:, 512:T], xt_d[0, 128 * d:128 * (d + 1), 512:T]
                )
            def emit_b1_xt():
                for d in range(ND):
                    nc.gpsimd.dma_start(
                        xts[(1, d)][:, :], xt_d[1, 128 * d:128 * (d + 1), :]
                    )

            qks, qklos, v1s, vts = {}, {}, {}, {}
            for b in range(BPC):
                qks[b] = qkpool.tile([128, T], BF16, tag="qk", name=f"qk{b}")
                qklos[b] = qklopool.tile([128, T], BF16, tag="qklo", name=f"qklo{b}")
                v1s[b] = v1pool.tile([128, NJ, HD + 1], BF16, tag="v1", name=f"v1{b}")
                vts[b] = vtpool.tile([64, T], F32, tag="vt", name=f"vt{b}")
            for b in range(BPC):
                nc.vector.tensor_copy(
                    v1s[b][:, :, HD:HD + 1],
                    onescol[:].rearrange("p (a c) -> p a c", c=1),
                )

            def proj_steps(b, Q):
                """Emission steps for everything attention chunk (b, Q) needs
                from tokens [512Q, 512Q+512): returns a list of callables so
                the caller can interleave them between attention pairs."""
                s = slice(512 * Q, 512 * (Q + 1))
                qk, qklo, v1, vt = qks[b], qklos[b], v1s[b], vts[b]
                state = {}

                def mk_qk(d):
                    def f():
                        if d == 0:
                            state["pqk"] = auxpool.tile(
                                [128, 512], F32, tag="aux", name="pqk"
                            )
                        nc.tensor.matmul(
                            state["pqk"][:], wqk[d], xts[(b, d)][:, s],
                            start=(d == 0), stop=(d == ND - 1),
                        )
                        if d == ND - 1:
                            nc.vector.tensor_copy(qk[:, s], state["pqk"][:])
                            # swapped halves: kT to 0:64, qT to 64:128
                            nc.sync.dma_start(qklo[0:64, s], qk[64:128, s])
                            nc.scalar.dma_start(qklo[64:128, s], qk[0:64, s])
                    return f

                def mk_v(d):
                    def f():
                        if d == 0:
                            state["pv"] = auxpool.tile(
                                [64, 512], F32, tag="aux", name="pv"
                            )
                        nc.tensor.matmul(
                            state["pv"][:], wv[d], xts[(b, d)][:, s],
                            start=(d == 0), stop=(d == ND - 1),
                        )
                        if d == ND - 1:
                            nc.vector.tensor_copy(vt[:, s], state["pv"][:])
                            for t2 in range(2 * Q, 2 * Q + 2):
                                p2 = auxpool.tile(
                                    [128, 128], F32, tag="aux", name="ptr"
                                )
                                for tt in range(2):
                                    nc.tensor.transpose(
                                        p2[:, 64 * tt:64 * (tt + 1)],
                                        vt[:, 128 * (2 * t2 + tt):
                                           128 * (2 * t2 + tt + 1)],
                                        ident[:],
                                    )
                                nc.vector.tensor_copy(
                                    v1[:, 2 * t2:2 * t2 + 2, 0:HD],
                                    p2[:].rearrange("p (a c) -> p a c", a=2),
                                )
                    return f

                return [mk_qk(d) for d in range(ND)] + [mk_v(d) for d in range(ND)]

            def emit_attn_q(b, Q, fillers):
                """One query chunk: all causal k-blocks in even/odd pairs
                run concurrently via 2-way PE row tiling; PV skewed one
                pair behind ST; proj steps for the next chunk interleaved."""
                qk, qklo, v1 = qks[b], qklos[b], v1s[b]
                pot = otpool.tile([HD + 1, 512], F32, tag="ot", name="pot")
                njb = 4 * (Q + 1)
                jlast = njb - 1

                def emit_pv(p_tile, g):
                    je, jo = 2 * g, 2 * g + 1
                    w0e = 128 * (je - 4 * Q) if je >= 4 * Q else 0
                    w0o = 128 * (jo - 4 * Q) if jo >= 4 * Q else 0
                    nc.tensor.matmul(
                        pot[:, w0e:512], v1[:, je, :], p_tile[:, w0e:512],
                        start=(je == 0), stop=False,
                    )
                    nc.tensor.matmul(
                        pot[:, w0o:512], v1[:, jo, :],
                        p_tile[:, 512 + w0o:1024],
                        start=False, stop=(jo == jlast),
                    )

                pending = []
                for g in range(njb // 2):
                    je, jo = 2 * g, 2 * g + 1
                    w0e = 128 * (je - 4 * Q) if je >= 4 * Q else 0
                    w0o = 128 * (jo - 4 * Q) if jo >= 4 * Q else 0
                    diag_e, diag_o = je >= 4 * Q, jo >= 4 * Q
                    pst = stpool.tile([128, 1024], F32, tag="st", name="pst")
                    # even block: kT stationary on partitions 0:64 (rg 0)
                    nc.tensor.matmul(
                        pst[:, w0e:512],
                        qklo[0:64, 128 * je:128 * (je + 1)],
                        qk[0:64, 512 * Q + w0e:512 * (Q + 1)],
                        start=True, stop=not diag_e,
                    )
                    # odd block: kT stationary on partitions 64:128 (rg 64)
                    nc.tensor.matmul(
                        pst[:, 512 + w0o:1024],
                        qk[64:128, 128 * jo:128 * (jo + 1)],
                        qklo[64:128, 512 * Q + w0o:512 * (Q + 1)],
                        start=True, stop=not diag_o,
                    )
                    # diagonal blocks: add -1e5 above the diagonal ON the PE
                    # (I.T @ mneg accumulated into the 128-wide diag window),
                    # so exp underflows to exactly 0 and no cross-engine mask
                    # op sits between exp and PV
                    if diag_e:
                        nc.tensor.matmul(
                            pst[:, w0e:w0e + 128], identb[:], mneg[:],
                            start=False, stop=True,
                        )
                    if diag_o:
                        nc.tensor.matmul(
                            pst[:, 512 + w0o:512 + w0o + 128], identb[:],
                            mneg[:], start=False, stop=True,
                        )
                    if fillers:
                        fillers.pop(0)()
                    ptil = ptpool.tile([128, 1024], BF16, tag="pt", name="ptil")
                    nc.scalar.activation(
                        ptil[:, w0e:1024], pst[:, w0e:1024], AF.Exp,
                        scale=1.0 / np.sqrt(HD),
                    )
                    # PV runs two pairs behind ST so the in-order PE stream
                    # never waits on the exp of the pair it just issued
                    pending.append((ptil, g))
                    if len(pending) > 2:
                        emit_pv(*pending.pop(0))
                for p in pending:
                    if fillers:
                        fillers.pop(0)()
                    emit_pv(*p)
                while fillers:
                    fillers.pop(0)()

                # normalize: out = oT[0:64] * broadcast(1 / l), PE-free
                # (reciprocal_approx_fast is a custom-DVE op that mishandles
                #  psum APs at a nonzero base partition — stage l via SBUF)
                lsb = recpool.tile([1, 512], F32, tag="lsb", name="lsb")
                nc.vector.tensor_copy(lsb[:], pot[HD:HD + 1, :])
                rsb = recpool.tile([1, 512], F32, tag="rsb", name="rsb")
                nc.vector.reciprocal_approx_fast(rsb[:], lsb[:])
                rcb = outpool.tile([HD, 512], F32, tag="rcb", name="rcb")
                nc.gpsimd.partition_broadcast(rcb[:], rsb[:])
                osb = outpool.tile([HD, 512], F32, tag="out", name="osb")
                nc.vector.tensor_mul(osb[:], pot[0:HD, :], rcb[:])
                nc.sync.dma_start(
                    ot_d[b, :, 512 * Q:512 * (Q + 1)], osb[:]
                )

            # ---- emission schedule (no fillers bisect) ----
            import os
            if os.environ.get("KNOFILL"):
                for b in range(BPC):
                    if b == 1:
                        emit_b1_xt()
                    for Q in range(NQ):
                        for f in proj_steps(b, Q):
                            f()
                        emit_attn_q(b, Q, [])
            else:
                for f in proj_steps(0, 0):
                    f()
                emit_attn_q(0, 0, proj_steps(0, 1))
                emit_attn_q(0, 1, proj_steps(0, 2))
                emit_b1_xt()
                emit_attn_q(0, 2, proj_steps(0, 3))
                emit_attn_q(0, 3, proj_steps(1, 0))
                emit_attn_q(1, 0, proj_steps(1, 1))
                emit_attn_q(1, 1, proj_steps(1, 2))
                emit_attn_q(1, 2, proj_steps(1, 3))
                emit_attn_q(1, 3, [])

    nc.compile()
    return nc


def _get_nc():
    if "nc" not in _cache:
        _cache["nc"] = _build_nc()
    return _cache["nc"]


def kernel(x, Wq, Wk, Wv, _trace=False, _trace_kwargs=None):
    import ml_dtypes
    from concourse.bass_utils import run_bass_kernel_spmd

    bf16 = ml_dtypes.bfloat16
    x = np.asarray(x, dtype=np.float32)
    Wq = np.asarray(Wq, dtype=np.float32)
    Wk = np.asarray(Wk, dtype=np.float32)
    Wv = np.asarray(Wv, dtype=np.float32)

    nc = _get_nc()

    wqk = np.ascontiguousarray(
        np.concatenate([Wq, Wk], axis=1).reshape(ND, 128, 128)
    ).astype(bf16)
    wv = np.ascontiguousarray(Wv.reshape(ND, 128, HD)).astype(bf16)
    mneg = np.where(
        np.arange(128)[None, :] < np.arange(128)[:, None], -1e5, 0.0
    ).astype(np.float32).astype(bf16)
    identb = np.eye(128, dtype=np.float32).astype(bf16)
    onescol = np.ones((128, NJ), dtype=np.float32).astype(bf16)
    ident = np.eye(64, dtype=np.float32)

    in_maps = []
    for c in range(NCORES):
        xt = np.ascontiguousarray(
            x[BPC * c:BPC * (c + 1)].transpose(0, 2, 1)
        ).astype(bf16)
        in_maps.append(
            {
                "xt": xt,
                "wqk": wqk,
                "wv": wv,
                "mneg": mneg,
                "identb": identb,
                "onescol": onescol,
                "ident": ident,
            }
        )

    kwargs = dict(_trace_kwargs or {})
    res = run_bass_kernel_spmd(
        nc, in_maps, list(range(NCORES)), trace=_trace, **kwargs
    )

    out = np.empty((B, T, HD), dtype=np.float32)
    for c in range(NCORES):
        ot = res.results[c]["ot"]  # [BPC, HD, T]
        out[BPC * c:BPC * (c + 1)] = ot.transpose(0, 2, 1)
    if _trace:
        _cache["last_results"] = res
    return out
